# revision 1
# baseline (speedup 1.0000x reference)
"""RGCN (segment_reduce) Trainium2 kernel v3 — 8 NeuronCores, full inputs in/out.

  - Degree norms computed on CPU, folded into per-edge scales that live in
    host-built scatter tiles S (bf16, DMA-shipped; same S serves both RGCN
    layers).
  - Edge tiles are r-merged: per (dst-block, half) sorted by (relation, src);
    a 128-edge gather tile feeds one scatter sub-matmul per relation-run.
  - Tables are [N,128] bf16 in AllGather-chunk-major order (host-remapped
    indices); AG chunks fire as their blocks flush, the last chunk is 14
    rows so the next layer starts almost immediately.
  - Flush orientation lhsT=agg_r, rhs=W_r gives [dst,hid] directly; biases
    enter via a rank-1 ones-matmul; everything bf16 except h0.
"""
import sys
import types

import numpy as np

if "antenv" not in sys.modules:
    try:
        import antenv  # noqa: F401
    except ImportError:
        _antenv = types.ModuleType("antenv")
        _antenv.__path__ = []
        sys.modules["antenv"] = _antenv

import concourse.bass as bass  # noqa: E402,F401
import concourse.bacc as bacc  # noqa: E402
import concourse.tile as tile  # noqa: E402
from concourse import mybir  # noqa: E402
import concourse.bass_utils as bass_utils  # noqa: E402

_DGE_ARGS = [
    "--dge-levels=scalar_dynamic_offset",
    "--dge-levels=vector_dynamic_offsets",
    "--dge-levels=dst_reduce",
]
if not getattr(bass_utils, "_dge_patched", False):
    _orig_run_command = bass_utils.run_command

    def _run_command_dge(argv, **kwargs):
        if argv and "walrus_driver" in str(argv[0]) and "--pass" in argv:
            argv = list(argv) + [a for a in _DGE_ARGS if a not in argv]
        return _orig_run_command(argv, **kwargs)

    bass_utils.run_command = _run_command_dge
    bass_utils._dge_patched = True

F32 = mybir.dt.float32
BF16 = mybir.dt.bfloat16
I16 = mybir.dt.int16
AF = mybir.ActivationFunctionType
ALU = mybir.AluOpType
NPBF16 = mybir.dt.np(BF16)

N_CORES = 8
P = 128
CHUNK_TILES = 8
CHUNK = CHUNK_TILES * P
SPLIT = 32768
# per-core shard row boundaries of the AllGather chunks (4096 must be a
# boundary: it maps to table row 32768 = the int16 gather-index split)
AG_EDGES = [0, 1024, 2048, 3072, 4096, 4750]


def _ceil(a, b):
    return -(-a // b)


class Struct:
    pass


# ---------------------------------------------------------------------------
# CPU-side prep
# ---------------------------------------------------------------------------
def _g_remap(n, shard):
    """node id -> row in the AG-chunk-major table layout."""
    n = np.asarray(n, np.int64)
    c = n // shard
    i = n - c * shard
    starts = np.asarray(AG_EDGES[:-1])
    ends = np.asarray(AG_EDGES[1:])
    k = np.searchsorted(ends, i, side="right")
    st = starts[k]
    sz = (ends - starts)[k]
    return N_CORES * st + c * sz + (i - st)


def _bin_by_dst(gsrc, dst, val, shard, n_blk):
    out = []
    for c in range(N_CORES):
        lo, hi = c * shard, (c + 1) * shard
        sel = (dst >= lo) & (dst < hi)
        ds = dst[sel] - lo
        gs = gsrc[sel]
        vs = val[sel]
        blk = ds // P
        order = np.lexsort((gs, blk))
        ds, gs, vs, blk = ds[order], gs[order], vs[order], blk[order]
        bounds = np.searchsorted(blk, np.arange(n_blk + 1))
        perblk = []
        for b in range(n_blk):
            sl = slice(bounds[b], bounds[b + 1])
            gb, db, vb = gs[sl], ds[sl] - b * P, vs[sl]
            m = gb < SPLIT
            perblk.append((gb[m], db[m], vb[m],
                           gb[~m] - SPLIT, db[~m], vb[~m]))
        out.append(perblk)
    return out


def _pack(groups_rc, n_blk, R):
    nt = np.zeros((n_blk, R, 2), np.int64)
    for r in range(R):
        for c in range(N_CORES):
            for b in range(n_blk):
                g = groups_rc[r][c][b]
                nt[b, r, 0] = max(nt[b, r, 0], _ceil(len(g[0]), P))
                nt[b, r, 1] = max(nt[b, r, 1], _ceil(len(g[3]), P))
    tiles = []
    tmap = {}
    for b in range(n_blk):
        for r in range(R):
            tot = int(nt[b, r, 0] + nt[b, r, 1])
            k = 0
            for half in (0, 1):
                for j in range(int(nt[b, r, half])):
                    tmap[(b, r, half, j)] = len(tiles)
                    tiles.append((b, r, half, k == 0, k == tot - 1))
                    k += 1
    NT = len(tiles)
    src16 = np.zeros((N_CORES, NT, P), np.int16)
    dloc = np.full((N_CORES, NT, P), -1, np.int64)
    dval = np.zeros((N_CORES, NT, P), np.float32)
    for c in range(N_CORES):
        for b in range(n_blk):
            for r in range(R):
                g = groups_rc[r][c][b]
                for half in (0, 1):
                    sarr = g[0] if half == 0 else g[3]
                    darr = g[1] if half == 0 else g[4]
                    varr = g[2] if half == 0 else g[5]
                    for j in range(_ceil(len(sarr), P)):
                        t = tmap[(b, r, half, j)]
                        seg = slice(j * P, (j + 1) * P)
                        n = len(sarr[seg])
                        src16[c, t, :n] = sarr[seg]
                        dloc[c, t, :n] = darr[seg]
                        dval[c, t, :n] = varr[seg]
    return tiles, src16, dloc, dval


def _chunks_of_tiles(tiles):
    lo = [i for i, t in enumerate(tiles) if t[2] == 0]
    hi = [i for i, t in enumerate(tiles) if t[2] == 1]
    chunks = []
    for half, stream in ((0, lo), (1, hi)):
        for i in range(0, len(stream), CHUNK_TILES):
            chunks.append((half, stream[i:i + CHUNK_TILES]))
    chunks.sort(key=lambda ch: min(ch[1]))
    slot = {}
    for ci, (_, tl) in enumerate(chunks):
        for j, t in enumerate(tl):
            slot[t] = (ci, j)
    return chunks, slot


def _wrap_idx(src16, chunks):
    ncore = src16.shape[0]
    out = np.zeros((ncore, 128, max(1, len(chunks)) * (CHUNK // 16)), np.int16)
    for ci, (_, tl) in enumerate(chunks):
        flat = np.zeros((ncore, CHUNK), np.int16)
        for j, t in enumerate(tl):
            flat[:, j * P:(j + 1) * P] = src16[:, t, :]
        out[:, :16, ci * 64:(ci + 1) * 64] = flat.reshape(
            ncore, CHUNK // 16, 16).transpose(0, 2, 1)
    out[:, 16:, :] = np.tile(out[:, :16, :], (1, 7, 1))
    return out


def prepare(inputs, cfg):
    s = Struct()
    s.cfg = cfg
    N, R, NROW = cfg["N"], cfg["R"], cfg["NROW"]
    shard, rshard = N // N_CORES, NROW // N_CORES
    n_blk, n_rblk = _ceil(shard, P), _ceil(rshard, P)
    s.shard, s.rshard, s.n_blk, s.n_rblk = shard, rshard, n_blk, n_rblk
    assert AG_EDGES[-1] == shard and AG_EDGES[4] * N_CORES == SPLIT

    es = np.asarray(inputs["edges_src"]).astype(np.int64)
    ed = np.asarray(inputs["edges_dst"]).astype(np.int64)

    g_main = []
    for r in range(R):
        deg_o = np.bincount(es[r], minlength=N).astype(np.float64)
        deg_i = np.bincount(ed[r], minlength=N).astype(np.float64)
        ns = np.where(deg_o > 0, deg_o, 1.0) ** -0.5
        nd = np.where(deg_i > 0, deg_i, 1.0) ** -0.5
        val = (ns[es[r]] * nd[ed[r]]).astype(np.float32)
        g_main.append(_bin_by_dst(_g_remap(es[r], shard), ed[r], val,
                                  shard, n_blk))
    s.tiles_e, src16_e, dloc_e, dval_e = _pack(g_main, n_blk, R)
    s.chunks_e, s.slot_e = _chunks_of_tiles(s.tiles_e)
    s.idx_e = _wrap_idx(src16_e, s.chunks_e)
    s.NT_e = len(s.tiles_e)
    s.n_ch_e = len(s.chunks_e)

    ridx = np.asarray(inputs["row_idx"]).astype(np.int64)
    rmask = np.asarray(inputs["row_mask"]).astype(bool)
    cnt = np.maximum(rmask.sum(1), 1).astype(np.float32)
    g_row = [[]]
    for c in range(N_CORES):
        lo = c * rshard
        rows = []
        for bb in range(n_rblk):
            i0 = lo + bb * P
            i1 = min(i0 + P, lo + rshard)
            ii, jj = np.nonzero(rmask[i0:i1])
            gs = _g_remap(ridx[i0:i1][ii, jj], shard)
            vv = (1.0 / cnt[i0 + ii]).astype(np.float32)
            order = np.argsort(gs, kind="stable")
            gs, ii2, vv = gs[order], ii[order], vv[order]
            m = gs < SPLIT
            rows.append((gs[m], ii2[m], vv[m],
                         gs[~m] - SPLIT, ii2[~m], vv[~m]))
        g_row[0].append(rows)
    s.tiles_r, src16_r, dloc_r, dval_r = _pack(g_row, n_rblk, 1)
    s.chunks_r, s.slot_r = _chunks_of_tiles(s.tiles_r)
    s.idx_r = _wrap_idx(src16_r, s.chunks_r)
    s.NT_r = len(s.tiles_r)
    s.n_ch_r = len(s.chunks_r)

    def build_S(NT, n_ch, slot, dloc, dval):
        out = []
        for c in range(N_CORES):
            Sh = np.zeros((P, n_ch * CHUNK), NPBF16)
            for ti in range(NT):
                ci, j = slot[ti]
                base = ci * CHUNK + j * P
                dl = dloc[c, ti]
                m = dl >= 0
                Sh[np.nonzero(m)[0], base + dl[m]] = \
                    dval[c, ti][m].astype(NPBF16)
            out.append(Sh)
        return out

    s.S_e = build_S(s.NT_e, s.n_ch_e, s.slot_e, dloc_e, dval_e)
    s.S_r = build_S(s.NT_r, s.n_ch_r, s.slot_r, dloc_r, dval_r)

    nf = np.asarray(inputs["node_feats"]).astype(np.float32)
    s.nfT_shards = []
    for c in range(N_CORES):
        m = np.zeros((n_blk * P, cfg["IN"]), np.float32)
        m[:shard] = nf[c * shard:(c + 1) * shard]
        s.nfT_shards.append(np.ascontiguousarray(m.T))
    return s


# ---------------------------------------------------------------------------
# device program
# ---------------------------------------------------------------------------
def build_program(s):
    cfg = s.cfg
    N, R = cfg["N"], cfg["R"]
    IN_D, HID, NCLS = cfg["IN"], cfg["HID"], cfg["NCLS"]
    n_blk, n_rblk, shard, rshard = s.n_blk, s.n_rblk, s.shard, s.rshard
    NQ = 4
    ag_start = AG_EDGES[:-1]
    ag_sizes = [AG_EDGES[i + 1] - AG_EDGES[i] for i in range(len(ag_start))]

    nc = bacc.Bacc("TRN2", target_bir_lowering=False, debug=False,
                   num_devices=N_CORES, num_swdge_queues=NQ)
    dp = nc.declare_dram_parameter
    t_nfT = dp("nfT", [IN_D, n_blk * P], F32, isOutput=False)
    t_Win = dp("W_in", [IN_D, HID], F32, isOutput=False)
    t_binrow = dp("binrow", [1, HID], F32, isOutput=False)
    t_W1 = dp("W1", [R * HID, HID], BF16, isOutput=False)
    t_W2 = dp("W2", [R * HID, HID], BF16, isOutput=False)
    t_bs1 = dp("bs1", [1, HID], BF16, isOutput=False)
    t_bs2 = dp("bs2", [1, HID], BF16, isOutput=False)
    t_Wm1 = dp("Wm1", [HID, HID], BF16, isOutput=False)
    t_Wm2 = dp("Wm2", [HID, HID], BF16, isOutput=False)
    t_Wm3 = dp("Wm3", [HID, NCLS], BF16, isOutput=False)
    t_bm1 = dp("bm1", [HID, 1], F32, isOutput=False)
    t_bm2 = dp("bm2", [HID, 1], F32, isOutput=False)
    t_bm3 = dp("bm3", [NCLS, 1], F32, isOutput=False)
    t_idx_e = dp("idx_e", list(s.idx_e.shape[1:]), I16, isOutput=False)
    t_idx_r = dp("idx_r", list(s.idx_r.shape[1:]), I16, isOutput=False)
    t_Se = dp("S_e", [128, s.n_ch_e * CHUNK], BF16, isOutput=False)
    t_Sr = dp("S_r", [128, s.n_ch_r * CHUNK], BF16, isOutput=False)
    t_out = dp("out", [NCLS, rshard], F32, isOutput=True)

    T = [nc.dram_tensor(f"T{i}", [N, HID], BF16, addr_space="Shared")
         for i in range(3)]
    Tsh = [nc.dram_tensor(f"T{i}sh", [shard, HID], BF16) for i in range(3)]

    with tile.TileContext(nc) as tc:
        import contextlib
        top = contextlib.ExitStack()
        kp = top.enter_context(tc.tile_pool(name="const", bufs=1))
        wp = top.enter_context(tc.tile_pool(name="weights", bufs=1))
        mp = top.enter_context(tc.tile_pool(name="meta", bufs=1))
        sb = top.enter_context(tc.tile_pool(name="sbwork", bufs=6))
        ttp = top.enter_context(tc.tile_pool(name="ttile", bufs=8))
        gp = top.enter_context(tc.tile_pool(name="gather", bufs=8))
        spool = top.enter_context(tc.tile_pool(name="spool", bufs=8))

        ones1 = kp.tile([1, 128], F32)
        nc.vector.memset(ones1[:], 1.0)
        ones1b = kp.tile([1, 128], BF16)
        nc.vector.memset(ones1b[:], 1.0)

        nfTsb = kp.tile([IN_D, n_blk * P], F32)
        nc.sync.dma_start(out=nfTsb[:], in_=t_nfT[:])
        Winsb = wp.tile([IN_D, HID], F32)
        nc.sync.dma_start(out=Winsb[:], in_=t_Win[:])
        binrow = wp.tile([1, HID], F32)
        nc.sync.dma_start(out=binrow[:], in_=t_binrow[:])
        W1sb = wp.tile([128, R * HID], BF16)
        W2sb = wp.tile([128, R * HID], BF16)
        for r in range(R):
            nc.sync.dma_start(out=W1sb[:HID, r * HID:(r + 1) * HID],
                              in_=t_W1[r * HID:(r + 1) * HID, :])
            nc.sync.dma_start(out=W2sb[:HID, r * HID:(r + 1) * HID],
                              in_=t_W2[r * HID:(r + 1) * HID, :])
        bs1row = wp.tile([1, HID], BF16)
        nc.sync.dma_start(out=bs1row[:], in_=t_bs1[:])
        bs2row = wp.tile([1, HID], BF16)
        nc.sync.dma_start(out=bs2row[:], in_=t_bs2[:])
        Wm1sb = wp.tile([HID, HID], BF16)
        nc.sync.dma_start(out=Wm1sb[:], in_=t_Wm1[:])
        Wm2sb = wp.tile([HID, HID], BF16)
        nc.sync.dma_start(out=Wm2sb[:], in_=t_Wm2[:])
        Wm3sb = wp.tile([HID, NCLS], BF16)
        nc.sync.dma_start(out=Wm3sb[:], in_=t_Wm3[:])
        bm1sb = wp.tile([HID, 1], F32)
        nc.sync.dma_start(out=bm1sb[:], in_=t_bm1[:])
        bm2sb = wp.tile([HID, 1], F32)
        nc.sync.dma_start(out=bm2sb[:], in_=t_bm2[:])
        bm3sb = wp.tile([NCLS, 1], F32)
        nc.sync.dma_start(out=bm3sb[:], in_=t_bm3[:])

        idxe_sb = mp.tile([128, s.idx_e.shape[2]], I16)
        nc.sync.dma_start(out=idxe_sb[:], in_=t_idx_e[:])
        idxr_sb = mp.tile([128, s.idx_r.shape[2]], I16)
        nc.sync.dma_start(out=idxr_sb[:], in_=t_idx_r[:])

        def write_table(l, b, src_ap, rows):
            tt = ttp.tile([128, HID], BF16, tag="tt")
            nc.scalar.activation(tt[:], src_ap,
                                 AF.Relu if l <= 1 else AF.Identity)
            nc.sync.dma_start(out=Tsh[l][b * P:b * P + rows, :],
                              in_=tt[:rows, :])

        def maybe_allgather(l, b):
            for k in range(len(ag_start)):
                if b == _ceil(AG_EDGES[k + 1], P) - 1:
                    nc.gpsimd.collective_compute(
                        "AllGather", ALU.bypass,
                        replica_groups=[list(range(N_CORES))],
                        ins=[Tsh[l][ag_start[k]:ag_start[k] + ag_sizes[k], :]],
                        outs=[T[l][N_CORES * ag_start[k]:
                                   N_CORES * (ag_start[k] + ag_sizes[k]), :]])

        # ---- phase 1: h0 = relu(x @ W_in + b_in) -------------------------
        with tc.tile_pool(name="ps_h0", bufs=4, space="PSUM") as pp:
            for b in range(n_blk):
                rows = min(P, shard - b * P)
                psh = pp.tile([128, HID], F32, tag="h0")
                nc.tensor.matmul(psh[:], lhsT=ones1[:], rhs=binrow[:],
                                 start=True, stop=False)
                nc.tensor.matmul(psh[:], lhsT=nfTsb[:, b * P:(b + 1) * P],
                                 rhs=Winsb[:], start=False, stop=True)
                write_table(0, b, psh[:], rows)
                maybe_allgather(0, b)

        # ---- phases 2-3: RGCN layers ------------------------------------
        def run_gathers(l, chunks, idx_sb):
            gtiles = {}
            for ci, (half, tl) in enumerate(chunks):
                g = gp.tile([128, CHUNK_TILES, HID], BF16, tag="g")
                src = T[l][0:SPLIT, :] if half == 0 else T[l][SPLIT:N, :]
                nc.gpsimd.dma_gather(
                    out_ap=g[:], in_ap=src,
                    idxs_ap=idx_sb[:, ci * 64:(ci + 1) * 64],
                    num_idxs=CHUNK, num_idxs_reg=CHUNK, elem_size=HID,
                    queue_num=ci % NQ)
                gtiles[ci] = g
            return gtiles

        def run_s_loads(t_S, n_ch):
            stiles = {}
            for ck in range(n_ch):
                St = spool.tile([128, CHUNK], BF16, tag="S")
                nc.sync.dma_start(out=St[:],
                                  in_=t_S[:, ck * CHUNK:(ck + 1) * CHUNK])
                stiles[ck] = St
            return stiles

        def run_layer(l):
            Wsb = W1sb if l == 0 else W2sb
            bsrow = bs1row if l == 0 else bs2row
            with (
                tc.tile_pool(name=f"psx{l}", bufs=2, space="PSUM") as psxp,
                tc.tile_pool(name=f"ps2{l}", bufs=2, space="PSUM") as ps2p,
            ):
                gtiles = run_gathers(l, s.chunks_e, idxe_sb)
                stiles = run_s_loads(t_Se, s.n_ch_e)

                def flush_block(b, psx, started):
                    rows = min(P, shard - b * P)
                    ps2 = ps2p.tile([128, 128], F32, tag="p2")
                    nc.tensor.matmul(ps2[:], lhsT=ones1b[:], rhs=bsrow[:],
                                     start=True, stop=False)
                    rlist = sorted(rr for (bb, rr) in started if bb == b)
                    for n_i, r in enumerate(rlist):
                        xs = sb.tile([128, 128], BF16, tag="xs")
                        nc.scalar.activation(xs[:], psx[:, r, :], AF.Copy)
                        nc.tensor.matmul(
                            ps2[:], lhsT=xs[:],
                            rhs=Wsb[:HID, r * HID:(r + 1) * HID],
                            start=False, stop=(n_i == len(rlist) - 1))
                    write_table(l + 1, b, ps2[:], rows)
                    maybe_allgather(l + 1, b)

                cur_blk, psx, started = -1, None, set()
                for ti, (b, r, half, first, last) in enumerate(s.tiles_e):
                    if b != cur_blk:
                        if cur_blk >= 0:
                            flush_block(cur_blk, psx, started)
                        cur_blk = b
                        psx = psxp.tile([128, R, 128], F32, tag="psx")
                        started = set()
                    ci, j = s.slot_e[ti]
                    nc.tensor.matmul(
                        psx[:, r, :], lhsT=gtiles[ci][:, j, :],
                        rhs=stiles[ci][:, j * P:(j + 1) * P],
                        start=(b, r) not in started, stop=last)
                    started.add((b, r))
                if cur_blk >= 0:
                    flush_block(cur_blk, psx, started)

        run_layer(0)
        run_layer(1)

        # ---- phase 4: rows + MLP ----------------------------------------
        with (
            tc.tile_pool(name="psr", bufs=2, space="PSUM") as psrp,
            tc.tile_pool(name="psm", bufs=2, space="PSUM") as psmp,
        ):
            gtiles = run_gathers(2, s.chunks_r, idxr_sb)
            stiles = run_s_loads(t_Sr, s.n_ch_r)

            def flush_rblock(bb, psr):
                xr = sb.tile([128, 128], BF16, tag="xr")
                nc.scalar.activation(xr[:], psr[:], AF.Copy)
                pm = psmp.tile([128, 128], F32, tag="pm")
                nc.tensor.matmul(pm[:], lhsT=Wm1sb[:], rhs=xr[:], start=True,
                                 stop=True)
                a1 = sb.tile([128, 128], BF16, tag="a1")
                nc.scalar.activation(a1[:], pm[:], AF.Relu, bias=bm1sb[:])
                pm2 = psmp.tile([128, 128], F32, tag="pm")
                nc.tensor.matmul(pm2[:], lhsT=Wm2sb[:], rhs=a1[:], start=True,
                                 stop=True)
                a2 = sb.tile([128, 128], BF16, tag="a2")
                nc.scalar.activation(a2[:], pm2[:], AF.Relu, bias=bm2sb[:])
                pm3 = psmp.tile([NCLS, 128], F32, tag="pm3")
                nc.tensor.matmul(pm3[:], lhsT=Wm3sb[:], rhs=a2[:], start=True,
                                 stop=True)
                ot = sb.tile([NCLS, 128], F32, tag="ot")
                nc.scalar.activation(ot[:], pm3[:], AF.Identity, bias=bm3sb[:])
                cols = min(P, rshard - bb * P)
                nc.sync.dma_start(out=t_out[:, bb * P:bb * P + cols],
                                  in_=ot[:, :cols])

            r_byblk = {}
            for ti, (bb, r0, half, first, last) in enumerate(s.tiles_r):
                r_byblk.setdefault(bb, []).append((ti, last))
            for bb in range(n_rblk):
                psr = psrp.tile([128, 128], F32, tag="psrT")
                tl = r_byblk.get(bb, [])
                if not tl:
                    nc.vector.memset(psr[:], 0.0)
                rstarted = False
                for (ti, last) in tl:
                    ci, j = s.slot_r[ti]
                    nc.tensor.matmul(psr[:], lhsT=gtiles[ci][:, j, :],
                                     rhs=stiles[ci][:, j * P:(j + 1) * P],
                                     start=not rstarted, stop=last)
                    rstarted = True
                flush_rblock(bb, psr)

        top.close()

    nc.compile()
    return nc


# ---------------------------------------------------------------------------
# entry point
# ---------------------------------------------------------------------------
def make_in_maps(inputs, s):
    cfg = s.cfg
    HID = cfg["HID"]
    W1 = np.asarray(inputs["W1"], np.float32).reshape(-1, HID).astype(NPBF16)
    W2 = np.asarray(inputs["W2"], np.float32).reshape(-1, HID).astype(NPBF16)
    bs1 = np.asarray(inputs["b1"], np.float32).sum(0).reshape(1, -1).astype(NPBF16)
    bs2 = np.asarray(inputs["b2"], np.float32).sum(0).reshape(1, -1).astype(NPBF16)
    in_maps = []
    for c in range(N_CORES):
        m = {
            "nfT": s.nfT_shards[c],
            "W_in": np.asarray(inputs["W_in"], np.float32),
            "binrow": np.asarray(inputs["b_in"], np.float32).reshape(1, -1),
            "W1": W1, "W2": W2, "bs1": bs1, "bs2": bs2,
            "Wm1": np.asarray(inputs["Wm1"], np.float32).astype(NPBF16),
            "Wm2": np.asarray(inputs["Wm2"], np.float32).astype(NPBF16),
            "Wm3": np.asarray(inputs["Wm3"], np.float32).astype(NPBF16),
            "bm1": np.asarray(inputs["bm1"], np.float32).reshape(-1, 1),
            "bm2": np.asarray(inputs["bm2"], np.float32).reshape(-1, 1),
            "bm3": np.asarray(inputs["bm3"], np.float32).reshape(-1, 1),
            "idx_e": np.ascontiguousarray(s.idx_e[c]),
            "idx_r": np.ascontiguousarray(s.idx_r[c]),
            "S_e": s.S_e[c],
            "S_r": s.S_r[c],
        }
        in_maps.append(m)
    return in_maps


def run(inputs, cfg, runner=None):
    s = prepare(inputs, cfg)
    nc = build_program(s)
    in_maps = make_in_maps(inputs, s)
    if runner is None:
        res = bass_utils.run_bass_kernel_spmd(nc, in_maps,
                                              core_ids=list(range(N_CORES)))
    else:
        res = runner(nc, in_maps)
    out = np.concatenate(
        [res.results[c]["out"][:, :s.rshard].T for c in range(N_CORES)],
        axis=0)
    return out.astype(np.float32), s, nc, res


def kernel(node_feats, edges_src, edges_dst, row_idx, row_mask,
           W_in, b_in, W1, b1, W2, b2, Wm1, bm1, Wm2, bm2, Wm3, bm3):
    cfg = dict(N=38000, R=8, NROW=60000, F=19, IN=64, HID=128, NCLS=10)
    inputs = dict(node_feats=node_feats, edges_src=edges_src,
                  edges_dst=edges_dst, row_idx=row_idx, row_mask=row_mask,
                  W_in=W_in, b_in=b_in, W1=W1, b1=b1, W2=W2, b2=b2,
                  Wm1=Wm1, bm1=bm1, Wm2=Wm2, bm2=bm2, Wm3=Wm3, bm3=bm3)
    out, _, _, _ = run(inputs, cfg)
    return out



# revision 10
# speedup vs baseline: 1.0127x; 1.0127x over previous
"""RGCN (segment_reduce) Trainium2 kernel v3 — 8 NeuronCores, full inputs in/out.

  - Degree norms computed on CPU, folded into per-edge scales that live in
    host-built scatter tiles S (bf16, DMA-shipped; same S serves both RGCN
    layers).
  - Edge tiles are r-merged: per (dst-block, half) sorted by (relation, src);
    a 128-edge gather tile feeds one scatter sub-matmul per relation-run.
  - Tables are [N,128] bf16 in AllGather-chunk-major order (host-remapped
    indices); AG chunks fire as their blocks flush, the last chunk is 14
    rows so the next layer starts almost immediately.
  - Flush orientation lhsT=agg_r, rhs=W_r gives [dst,hid] directly; biases
    enter via a rank-1 ones-matmul; everything bf16 except h0.
"""
import sys
import types

import numpy as np

if "antenv" not in sys.modules:
    try:
        import antenv  # noqa: F401
    except ImportError:
        _antenv = types.ModuleType("antenv")
        _antenv.__path__ = []
        sys.modules["antenv"] = _antenv

import concourse.bass as bass  # noqa: E402,F401
import concourse.bacc as bacc  # noqa: E402
import concourse.tile as tile  # noqa: E402
from concourse import mybir  # noqa: E402
import concourse.bass_utils as bass_utils  # noqa: E402

_DGE_ARGS = [
    "--dge-levels=scalar_dynamic_offset",
    "--dge-levels=vector_dynamic_offsets",
    "--dge-levels=dst_reduce",
]
if not getattr(bass_utils, "_dge_patched", False):
    _orig_run_command = bass_utils.run_command

    def _run_command_dge(argv, **kwargs):
        if argv and "walrus_driver" in str(argv[0]) and "--pass" in argv:
            argv = list(argv) + [a for a in _DGE_ARGS if a not in argv]
        return _orig_run_command(argv, **kwargs)

    bass_utils.run_command = _run_command_dge
    bass_utils._dge_patched = True

F32 = mybir.dt.float32
BF16 = mybir.dt.bfloat16
I16 = mybir.dt.int16
AF = mybir.ActivationFunctionType
ALU = mybir.AluOpType
NPBF16 = mybir.dt.np(BF16)

N_CORES = 8
P = 128
CHUNK_TILES = 8
CHUNK = CHUNK_TILES * P
SPLIT = 32768
# per-core shard row boundaries of the AllGather chunks (4096 must be a
# boundary: it maps to table row 32768 = the int16 gather-index split)
AG_EDGES = [0, 1024, 2048, 3072, 4096, 4750]


def _ceil(a, b):
    return -(-a // b)


class Struct:
    pass


# ---------------------------------------------------------------------------
# CPU-side prep
# ---------------------------------------------------------------------------
def _g_remap(n, shard):
    """node id -> row in the AG-chunk-major table layout."""
    n = np.asarray(n, np.int64)
    c = n // shard
    i = n - c * shard
    starts = np.asarray(AG_EDGES[:-1])
    ends = np.asarray(AG_EDGES[1:])
    k = np.searchsorted(ends, i, side="right")
    st = starts[k]
    sz = (ends - starts)[k]
    return N_CORES * st + c * sz + (i - st)


def _bin_by_dst(gsrc, dst, val, shard, n_blk):
    out = []
    for c in range(N_CORES):
        lo, hi = c * shard, (c + 1) * shard
        sel = (dst >= lo) & (dst < hi)
        ds = dst[sel] - lo
        gs = gsrc[sel]
        vs = val[sel]
        blk = ds // P
        order = np.lexsort((gs, blk))
        ds, gs, vs, blk = ds[order], gs[order], vs[order], blk[order]
        bounds = np.searchsorted(blk, np.arange(n_blk + 1))
        perblk = []
        for b in range(n_blk):
            sl = slice(bounds[b], bounds[b + 1])
            gb, db, vb = gs[sl], ds[sl] - b * P, vs[sl]
            m = gb < SPLIT
            perblk.append((gb[m], db[m], vb[m],
                           gb[~m] - SPLIT, db[~m], vb[~m]))
        out.append(perblk)
    return out


def _pack(groups_rc, n_blk, R):
    nt = np.zeros((n_blk, R, 2), np.int64)
    for r in range(R):
        for c in range(N_CORES):
            for b in range(n_blk):
                g = groups_rc[r][c][b]
                nt[b, r, 0] = max(nt[b, r, 0], _ceil(len(g[0]), P))
                nt[b, r, 1] = max(nt[b, r, 1], _ceil(len(g[3]), P))
    tiles = []
    tmap = {}
    for b in range(n_blk):
        for r in range(R):
            tot = int(nt[b, r, 0] + nt[b, r, 1])
            k = 0
            for half in (0, 1):
                for j in range(int(nt[b, r, half])):
                    tmap[(b, r, half, j)] = len(tiles)
                    tiles.append((b, r, half, k == 0, k == tot - 1))
                    k += 1
    NT = len(tiles)
    src16 = np.zeros((N_CORES, NT, P), np.int16)
    dloc = np.full((N_CORES, NT, P), -1, np.int64)
    dval = np.zeros((N_CORES, NT, P), np.float32)
    for c in range(N_CORES):
        for b in range(n_blk):
            for r in range(R):
                g = groups_rc[r][c][b]
                for half in (0, 1):
                    sarr = g[0] if half == 0 else g[3]
                    darr = g[1] if half == 0 else g[4]
                    varr = g[2] if half == 0 else g[5]
                    for j in range(_ceil(len(sarr), P)):
                        t = tmap[(b, r, half, j)]
                        seg = slice(j * P, (j + 1) * P)
                        n = len(sarr[seg])
                        src16[c, t, :n] = sarr[seg]
                        dloc[c, t, :n] = darr[seg]
                        dval[c, t, :n] = varr[seg]
    return tiles, src16, dloc, dval


def _chunks_of_tiles(tiles):
    lo = [i for i, t in enumerate(tiles) if t[2] == 0]
    hi = [i for i, t in enumerate(tiles) if t[2] == 1]
    chunks = []
    for half, stream in ((0, lo), (1, hi)):
        for i in range(0, len(stream), CHUNK_TILES):
            chunks.append((half, stream[i:i + CHUNK_TILES]))
    chunks.sort(key=lambda ch: min(ch[1]))
    slot = {}
    for ci, (_, tl) in enumerate(chunks):
        for j, t in enumerate(tl):
            slot[t] = (ci, j)
    return chunks, slot


def _wrap_idx(src16, chunks):
    ncore = src16.shape[0]
    out = np.zeros((ncore, 128, max(1, len(chunks)) * (CHUNK // 16)), np.int16)
    for ci, (_, tl) in enumerate(chunks):
        flat = np.zeros((ncore, CHUNK), np.int16)
        for j, t in enumerate(tl):
            flat[:, j * P:(j + 1) * P] = src16[:, t, :]
        out[:, :16, ci * 64:(ci + 1) * 64] = flat.reshape(
            ncore, CHUNK // 16, 16).transpose(0, 2, 1)
    out[:, 16:, :] = np.tile(out[:, :16, :], (1, 7, 1))
    return out


def prepare(inputs, cfg):
    s = Struct()
    s.cfg = cfg
    N, R, NROW = cfg["N"], cfg["R"], cfg["NROW"]
    shard, rshard = N // N_CORES, NROW // N_CORES
    n_blk, n_rblk = _ceil(shard, P), _ceil(rshard, P)
    s.shard, s.rshard, s.n_blk, s.n_rblk = shard, rshard, n_blk, n_rblk
    assert AG_EDGES[-1] == shard and AG_EDGES[4] * N_CORES == SPLIT

    es = np.asarray(inputs["edges_src"]).astype(np.int64)
    ed = np.asarray(inputs["edges_dst"]).astype(np.int64)

    g_main = []
    for r in range(R):
        deg_o = np.bincount(es[r], minlength=N).astype(np.float64)
        deg_i = np.bincount(ed[r], minlength=N).astype(np.float64)
        ns = np.where(deg_o > 0, deg_o, 1.0) ** -0.5
        nd = np.where(deg_i > 0, deg_i, 1.0) ** -0.5
        val = (ns[es[r]] * nd[ed[r]]).astype(np.float32)
        g_main.append(_bin_by_dst(_g_remap(es[r], shard), ed[r], val,
                                  shard, n_blk))
    s.tiles_e, src16_e, dloc_e, dval_e = _pack(g_main, n_blk, R)
    s.chunks_e, s.slot_e = _chunks_of_tiles(s.tiles_e)
    s.idx_e = _wrap_idx(src16_e, s.chunks_e)
    s.NT_e = len(s.tiles_e)
    s.n_ch_e = len(s.chunks_e)

    ridx = np.asarray(inputs["row_idx"]).astype(np.int64)
    rmask = np.asarray(inputs["row_mask"]).astype(bool)
    cnt = np.maximum(rmask.sum(1), 1).astype(np.float32)
    g_row = [[]]
    for c in range(N_CORES):
        lo = c * rshard
        rows = []
        for bb in range(n_rblk):
            i0 = lo + bb * P
            i1 = min(i0 + P, lo + rshard)
            ii, jj = np.nonzero(rmask[i0:i1])
            gs = _g_remap(ridx[i0:i1][ii, jj], shard)
            vv = (1.0 / cnt[i0 + ii]).astype(np.float32)
            order = np.argsort(gs, kind="stable")
            gs, ii2, vv = gs[order], ii[order], vv[order]
            m = gs < SPLIT
            rows.append((gs[m], ii2[m], vv[m],
                         gs[~m] - SPLIT, ii2[~m], vv[~m]))
        g_row[0].append(rows)
    s.tiles_r, src16_r, dloc_r, dval_r = _pack(g_row, n_rblk, 1)
    s.chunks_r, s.slot_r = _chunks_of_tiles(s.tiles_r)
    s.idx_r = _wrap_idx(src16_r, s.chunks_r)
    s.NT_r = len(s.tiles_r)
    s.n_ch_r = len(s.chunks_r)

    def build_S(NT, n_ch, slot, dloc, dval):
        out = []
        for c in range(N_CORES):
            Sh = np.zeros((P, n_ch * CHUNK), NPBF16)
            for ti in range(NT):
                ci, j = slot[ti]
                base = ci * CHUNK + j * P
                dl = dloc[c, ti]
                m = dl >= 0
                Sh[np.nonzero(m)[0], base + dl[m]] = \
                    dval[c, ti][m].astype(NPBF16)
            out.append(Sh)
        return out

    s.S_e = build_S(s.NT_e, s.n_ch_e, s.slot_e, dloc_e, dval_e)
    s.S_r = build_S(s.NT_r, s.n_ch_r, s.slot_r, dloc_r, dval_r)

    nf = np.asarray(inputs["node_feats"]).astype(np.float32)
    s.nfT_shards = []
    for c in range(N_CORES):
        m = np.zeros((n_blk * P, cfg["IN"]), np.float32)
        m[:shard] = nf[c * shard:(c + 1) * shard]
        s.nfT_shards.append(np.ascontiguousarray(m.T))
    return s


# ---------------------------------------------------------------------------
# device program
# ---------------------------------------------------------------------------
def build_program(s):
    cfg = s.cfg
    N, R = cfg["N"], cfg["R"]
    IN_D, HID, NCLS = cfg["IN"], cfg["HID"], cfg["NCLS"]
    n_blk, n_rblk, shard, rshard = s.n_blk, s.n_rblk, s.shard, s.rshard
    NQ = 4
    ag_start = AG_EDGES[:-1]
    ag_sizes = [AG_EDGES[i + 1] - AG_EDGES[i] for i in range(len(ag_start))]

    nc = bacc.Bacc("TRN2", target_bir_lowering=False, debug=False,
                   num_devices=N_CORES, num_swdge_queues=NQ)
    dp = nc.declare_dram_parameter
    t_nfT = dp("nfT", [IN_D, n_blk * P], F32, isOutput=False)
    t_Win = dp("W_in", [IN_D, HID], F32, isOutput=False)
    t_binrow = dp("binrow", [1, HID], F32, isOutput=False)
    t_W1 = dp("W1", [R * HID, HID], BF16, isOutput=False)
    t_W2 = dp("W2", [R * HID, HID], BF16, isOutput=False)
    t_bs1 = dp("bs1", [1, HID], BF16, isOutput=False)
    t_bs2 = dp("bs2", [1, HID], BF16, isOutput=False)
    t_Wm1 = dp("Wm1", [HID, HID], BF16, isOutput=False)
    t_Wm2 = dp("Wm2", [HID, HID], BF16, isOutput=False)
    t_Wm3 = dp("Wm3", [HID, NCLS], BF16, isOutput=False)
    t_bm1 = dp("bm1", [HID, 1], F32, isOutput=False)
    t_bm2 = dp("bm2", [HID, 1], F32, isOutput=False)
    t_bm3 = dp("bm3", [NCLS, 1], F32, isOutput=False)
    t_idx_e = dp("idx_e", list(s.idx_e.shape[1:]), I16, isOutput=False)
    t_idx_r = dp("idx_r", list(s.idx_r.shape[1:]), I16, isOutput=False)
    t_Se = dp("S_e", [128, s.n_ch_e * CHUNK], BF16, isOutput=False)
    t_Sr = dp("S_r", [128, s.n_ch_r * CHUNK], BF16, isOutput=False)
    t_out = dp("out", [NCLS, rshard], F32, isOutput=True)

    T = [nc.dram_tensor(f"T{i}", [N, HID], BF16, addr_space="Shared")
         for i in range(3)]
    Tsh = [nc.dram_tensor(f"T{i}sh", [shard, HID], BF16) for i in range(3)]

    with tile.TileContext(nc) as tc:
        import contextlib
        top = contextlib.ExitStack()
        kp = top.enter_context(tc.tile_pool(name="const", bufs=1))
        wp = top.enter_context(tc.tile_pool(name="weights", bufs=1))
        mp = top.enter_context(tc.tile_pool(name="meta", bufs=1))
        sb = top.enter_context(tc.tile_pool(name="sbwork", bufs=6))
        ttp = top.enter_context(tc.tile_pool(name="ttile", bufs=8))
        gp = top.enter_context(tc.tile_pool(name="gather", bufs=10))
        spool = top.enter_context(tc.tile_pool(name="spool", bufs=10))

        ones1 = kp.tile([1, 128], F32)
        nc.vector.memset(ones1[:], 1.0)
        ones1b = kp.tile([1, 128], BF16)
        nc.vector.memset(ones1b[:], 1.0)

        nfTsb = kp.tile([IN_D, n_blk * P], F32)
        nc.sync.dma_start(out=nfTsb[:], in_=t_nfT[:])
        Winsb = wp.tile([IN_D, HID], F32)
        nc.sync.dma_start(out=Winsb[:], in_=t_Win[:])
        binrow = wp.tile([1, HID], F32)
        nc.sync.dma_start(out=binrow[:], in_=t_binrow[:])
        W1sb = wp.tile([128, R * HID], BF16)
        W2sb = wp.tile([128, R * HID], BF16)
        for r in range(R):
            nc.sync.dma_start(out=W1sb[:HID, r * HID:(r + 1) * HID],
                              in_=t_W1[r * HID:(r + 1) * HID, :])
            nc.sync.dma_start(out=W2sb[:HID, r * HID:(r + 1) * HID],
                              in_=t_W2[r * HID:(r + 1) * HID, :])
        bs1row = wp.tile([1, HID], BF16)
        nc.sync.dma_start(out=bs1row[:], in_=t_bs1[:])
        bs2row = wp.tile([1, HID], BF16)
        nc.sync.dma_start(out=bs2row[:], in_=t_bs2[:])
        Wm1sb = wp.tile([HID, HID], BF16)
        nc.sync.dma_start(out=Wm1sb[:], in_=t_Wm1[:])
        Wm2sb = wp.tile([HID, HID], BF16)
        nc.sync.dma_start(out=Wm2sb[:], in_=t_Wm2[:])
        Wm3sb = wp.tile([HID, NCLS], BF16)
        nc.sync.dma_start(out=Wm3sb[:], in_=t_Wm3[:])
        bm1sb = wp.tile([HID, 1], F32)
        nc.sync.dma_start(out=bm1sb[:], in_=t_bm1[:])
        bm2sb = wp.tile([HID, 1], F32)
        nc.sync.dma_start(out=bm2sb[:], in_=t_bm2[:])
        bm3sb = wp.tile([NCLS, 1], F32)
        nc.sync.dma_start(out=bm3sb[:], in_=t_bm3[:])

        idxe_sb = mp.tile([128, s.idx_e.shape[2]], I16)
        nc.sync.dma_start(out=idxe_sb[:], in_=t_idx_e[:])
        idxr_sb = mp.tile([128, s.idx_r.shape[2]], I16)
        nc.sync.dma_start(out=idxr_sb[:], in_=t_idx_r[:])

        def write_table(l, b, src_ap, rows):
            tt = ttp.tile([128, HID], BF16, tag="tt")
            nc.scalar.activation(tt[:], src_ap,
                                 AF.Relu if l <= 1 else AF.Identity)
            nc.scalar.dma_start(out=Tsh[l][b * P:b * P + rows, :],
                                in_=tt[:rows, :])

        def maybe_allgather(l, b):
            for k in range(len(ag_start)):
                if b == _ceil(AG_EDGES[k + 1], P) - 1:
                    nc.gpsimd.collective_compute(
                        "AllGather", ALU.bypass,
                        replica_groups=[list(range(N_CORES))],
                        ins=[Tsh[l][ag_start[k]:ag_start[k] + ag_sizes[k], :]],
                        outs=[T[l][N_CORES * ag_start[k]:
                                   N_CORES * (ag_start[k] + ag_sizes[k]), :]])

        # ---- phase 1: h0 = relu(x @ W_in + b_in) -------------------------
        with tc.tile_pool(name="ps_h0", bufs=4, space="PSUM") as pp:
            for b in range(n_blk):
                rows = min(P, shard - b * P)
                psh = pp.tile([128, HID], F32, tag="h0")
                nc.tensor.matmul(psh[:], lhsT=ones1[:], rhs=binrow[:],
                                 start=True, stop=False)
                nc.tensor.matmul(psh[:], lhsT=nfTsb[:, b * P:(b + 1) * P],
                                 rhs=Winsb[:], start=False, stop=True)
                write_table(0, b, psh[:], rows)
                maybe_allgather(0, b)

        # ---- phases 2-3: RGCN layers ------------------------------------
        LOOKAHEAD = 7

        def lazy_chunks(l, chunks, idx_sb, t_S):
            """JIT gather + S-load emission so AGs/writes interleave."""
            gtiles, stiles, state = {}, {}, {"next": 0}

            def ensure(ci):
                tgt = min(ci + LOOKAHEAD, len(chunks) - 1)
                while state["next"] <= tgt:
                    c2 = state["next"]
                    half, _tl = chunks[c2]
                    g = gp.tile([128, CHUNK_TILES, HID], BF16, tag="g")
                    src = T[l][0:SPLIT, :] if half == 0 else T[l][SPLIT:N, :]
                    nc.gpsimd.dma_gather(
                        out_ap=g[:], in_ap=src,
                        idxs_ap=idx_sb[:, c2 * 64:(c2 + 1) * 64],
                        num_idxs=CHUNK, num_idxs_reg=CHUNK, elem_size=HID,
                        queue_num=c2 % NQ)
                    gtiles[c2] = g
                    St = spool.tile([128, CHUNK], BF16, tag="S")
                    nc.sync.dma_start(out=St[:],
                                      in_=t_S[:, c2 * CHUNK:(c2 + 1) * CHUNK])
                    stiles[c2] = St
                    state["next"] += 1
            return gtiles, stiles, ensure

        def run_layer(l):
            Wsb = W1sb if l == 0 else W2sb
            bsrow = bs1row if l == 0 else bs2row
            with (
                tc.tile_pool(name=f"psx{l}", bufs=2, space="PSUM") as psxp,
                tc.tile_pool(name=f"ps2{l}", bufs=2, space="PSUM") as ps2p,
            ):
                gtiles, stiles, ensure = lazy_chunks(l, s.chunks_e,
                                                     idxe_sb, t_Se)

                def flush_block(b, psx, started):
                    rows = min(P, shard - b * P)
                    ps2 = ps2p.tile([128, 128], F32, tag="p2")
                    nc.tensor.matmul(ps2[:], lhsT=ones1b[:], rhs=bsrow[:],
                                     start=True, stop=False)
                    rlist = sorted(rr for (bb, rr) in started if bb == b)
                    for n_i, r in enumerate(rlist):
                        xs = sb.tile([128, 128], BF16, tag="xs")
                        nc.scalar.activation(xs[:], psx[:, r, :], AF.Copy)
                        nc.tensor.matmul(
                            ps2[:], lhsT=xs[:],
                            rhs=Wsb[:HID, r * HID:(r + 1) * HID],
                            start=False, stop=(n_i == len(rlist) - 1))
                    write_table(l + 1, b, ps2[:], rows)
                    maybe_allgather(l + 1, b)

                cur_blk, psx, started = -1, None, set()
                for ti, (b, r, half, first, last) in enumerate(s.tiles_e):
                    if b != cur_blk:
                        if cur_blk >= 0:
                            flush_block(cur_blk, psx, started)
                        cur_blk = b
                        psx = psxp.tile([128, R, 128], F32, tag="psx")
                        started = set()
                    ci, j = s.slot_e[ti]
                    ensure(ci)
                    nc.tensor.matmul(
                        psx[:, r, :], lhsT=gtiles[ci][:, j, :],
                        rhs=stiles[ci][:, j * P:(j + 1) * P],
                        start=(b, r) not in started, stop=last)
                    started.add((b, r))
                if cur_blk >= 0:
                    flush_block(cur_blk, psx, started)

        run_layer(0)
        run_layer(1)

        # ---- phase 4: rows + MLP ----------------------------------------
        with (
            tc.tile_pool(name="psr", bufs=2, space="PSUM") as psrp,
            tc.tile_pool(name="psm", bufs=2, space="PSUM") as psmp,
        ):
            gtiles, stiles, ensure = lazy_chunks(2, s.chunks_r,
                                                 idxr_sb, t_Sr)

            def flush_rblock(bb, psr):
                xr = sb.tile([128, 128], BF16, tag="xr")
                nc.scalar.activation(xr[:], psr[:], AF.Copy)
                pm = psmp.tile([128, 128], F32, tag="pm")
                nc.tensor.matmul(pm[:], lhsT=Wm1sb[:], rhs=xr[:], start=True,
                                 stop=True)
                a1 = sb.tile([128, 128], BF16, tag="a1")
                nc.scalar.activation(a1[:], pm[:], AF.Relu, bias=bm1sb[:])
                pm2 = psmp.tile([128, 128], F32, tag="pm")
                nc.tensor.matmul(pm2[:], lhsT=Wm2sb[:], rhs=a1[:], start=True,
                                 stop=True)
                a2 = sb.tile([128, 128], BF16, tag="a2")
                nc.scalar.activation(a2[:], pm2[:], AF.Relu, bias=bm2sb[:])
                pm3 = psmp.tile([NCLS, 128], F32, tag="pm3")
                nc.tensor.matmul(pm3[:], lhsT=Wm3sb[:], rhs=a2[:], start=True,
                                 stop=True)
                ot = sb.tile([NCLS, 128], F32, tag="ot")
                nc.scalar.activation(ot[:], pm3[:], AF.Identity, bias=bm3sb[:])
                cols = min(P, rshard - bb * P)
                nc.scalar.dma_start(out=t_out[:, bb * P:bb * P + cols],
                                    in_=ot[:, :cols])

            r_byblk = {}
            for ti, (bb, r0, half, first, last) in enumerate(s.tiles_r):
                r_byblk.setdefault(bb, []).append((ti, last))
            for bb in range(n_rblk):
                psr = psrp.tile([128, 128], F32, tag="psrT")
                tl = r_byblk.get(bb, [])
                if not tl:
                    nc.vector.memset(psr[:], 0.0)
                rstarted = False
                for (ti, last) in tl:
                    ci, j = s.slot_r[ti]
                    ensure(ci)
                    nc.tensor.matmul(psr[:], lhsT=gtiles[ci][:, j, :],
                                     rhs=stiles[ci][:, j * P:(j + 1) * P],
                                     start=not rstarted, stop=last)
                    rstarted = True
                flush_rblock(bb, psr)

        top.close()

    nc.compile()
    return nc


# ---------------------------------------------------------------------------
# entry point
# ---------------------------------------------------------------------------
def make_in_maps(inputs, s):
    cfg = s.cfg
    HID = cfg["HID"]
    W1 = np.asarray(inputs["W1"], np.float32).reshape(-1, HID).astype(NPBF16)
    W2 = np.asarray(inputs["W2"], np.float32).reshape(-1, HID).astype(NPBF16)
    bs1 = np.asarray(inputs["b1"], np.float32).sum(0).reshape(1, -1).astype(NPBF16)
    bs2 = np.asarray(inputs["b2"], np.float32).sum(0).reshape(1, -1).astype(NPBF16)
    in_maps = []
    for c in range(N_CORES):
        m = {
            "nfT": s.nfT_shards[c],
            "W_in": np.asarray(inputs["W_in"], np.float32),
            "binrow": np.asarray(inputs["b_in"], np.float32).reshape(1, -1),
            "W1": W1, "W2": W2, "bs1": bs1, "bs2": bs2,
            "Wm1": np.asarray(inputs["Wm1"], np.float32).astype(NPBF16),
            "Wm2": np.asarray(inputs["Wm2"], np.float32).astype(NPBF16),
            "Wm3": np.asarray(inputs["Wm3"], np.float32).astype(NPBF16),
            "bm1": np.asarray(inputs["bm1"], np.float32).reshape(-1, 1),
            "bm2": np.asarray(inputs["bm2"], np.float32).reshape(-1, 1),
            "bm3": np.asarray(inputs["bm3"], np.float32).reshape(-1, 1),
            "idx_e": np.ascontiguousarray(s.idx_e[c]),
            "idx_r": np.ascontiguousarray(s.idx_r[c]),
            "S_e": s.S_e[c],
            "S_r": s.S_r[c],
        }
        in_maps.append(m)
    return in_maps


def run(inputs, cfg, runner=None):
    s = prepare(inputs, cfg)
    nc = build_program(s)
    in_maps = make_in_maps(inputs, s)
    if runner is None:
        res = bass_utils.run_bass_kernel_spmd(nc, in_maps,
                                              core_ids=list(range(N_CORES)))
    else:
        res = runner(nc, in_maps)
    out = np.concatenate(
        [res.results[c]["out"][:, :s.rshard].T for c in range(N_CORES)],
        axis=0)
    return out.astype(np.float32), s, nc, res


def kernel(node_feats, edges_src, edges_dst, row_idx, row_mask,
           W_in, b_in, W1, b1, W2, b2, Wm1, bm1, Wm2, bm2, Wm3, bm3):
    cfg = dict(N=38000, R=8, NROW=60000, F=19, IN=64, HID=128, NCLS=10)
    inputs = dict(node_feats=node_feats, edges_src=edges_src,
                  edges_dst=edges_dst, row_idx=row_idx, row_mask=row_mask,
                  W_in=W_in, b_in=b_in, W1=W1, b1=b1, W2=W2, b2=b2,
                  Wm1=Wm1, bm1=bm1, Wm2=Wm2, bm2=bm2, Wm3=Wm3, bm3=bm3)
    out, _, _, _ = run(inputs, cfg)
    return out



# revision 23
# speedup vs baseline: 1.0133x; 1.0005x over previous
"""RGCN (segment_reduce) Trainium2 kernel v3 — 8 NeuronCores, full inputs in/out.

  - Degree norms computed on CPU, folded into per-edge scales that live in
    host-built scatter tiles S (bf16, DMA-shipped; same S serves both RGCN
    layers).
  - Edge tiles are r-merged: per (dst-block, half) sorted by (relation, src);
    a 128-edge gather tile feeds one scatter sub-matmul per relation-run.
  - Tables are [N,128] bf16 in AllGather-chunk-major order (host-remapped
    indices); AG chunks fire as their blocks flush, the last chunk is 14
    rows so the next layer starts almost immediately.
  - Flush orientation lhsT=agg_r, rhs=W_r gives [dst,hid] directly; biases
    enter via a rank-1 ones-matmul; everything bf16 except h0.
"""
import sys
import types

import numpy as np

if "antenv" not in sys.modules:
    try:
        import antenv  # noqa: F401
    except ImportError:
        _antenv = types.ModuleType("antenv")
        _antenv.__path__ = []
        sys.modules["antenv"] = _antenv

import concourse.bass as bass  # noqa: E402,F401
import concourse.bacc as bacc  # noqa: E402
import concourse.tile as tile  # noqa: E402
from concourse import mybir  # noqa: E402
import concourse.bass_utils as bass_utils  # noqa: E402

_DGE_ARGS = [
    "--dge-levels=scalar_dynamic_offset",
    "--dge-levels=vector_dynamic_offsets",
    "--dge-levels=dst_reduce",
]
if not getattr(bass_utils, "_dge_patched", False):
    _orig_run_command = bass_utils.run_command

    def _run_command_dge(argv, **kwargs):
        if argv and "walrus_driver" in str(argv[0]) and "--pass" in argv:
            argv = list(argv) + [a for a in _DGE_ARGS if a not in argv]
        return _orig_run_command(argv, **kwargs)

    bass_utils.run_command = _run_command_dge
    bass_utils._dge_patched = True

F32 = mybir.dt.float32
BF16 = mybir.dt.bfloat16
I16 = mybir.dt.int16
AF = mybir.ActivationFunctionType
ALU = mybir.AluOpType
NPBF16 = mybir.dt.np(BF16)

N_CORES = 8
P = 128
CHUNK_TILES = 8
CHUNK = CHUNK_TILES * P
SPLIT = 32768
# per-core shard row boundaries of the AllGather chunks (4096 must be a
# boundary: it maps to table row 32768 = the int16 gather-index split)
AG_EDGES = [0, 1024, 2048, 3072, 4096, 4736, 4750]


def _ceil(a, b):
    return -(-a // b)


class Struct:
    pass


# ---------------------------------------------------------------------------
# CPU-side prep
# ---------------------------------------------------------------------------
def _g_remap(n, shard):
    """node id -> row in the AG-chunk-major table layout."""
    n = np.asarray(n, np.int64)
    c = n // shard
    i = n - c * shard
    starts = np.asarray(AG_EDGES[:-1])
    ends = np.asarray(AG_EDGES[1:])
    k = np.searchsorted(ends, i, side="right")
    st = starts[k]
    sz = (ends - starts)[k]
    return N_CORES * st + c * sz + (i - st)


def _bin_by_dst(gsrc, dst, val, shard, n_blk):
    out = []
    for c in range(N_CORES):
        lo, hi = c * shard, (c + 1) * shard
        sel = (dst >= lo) & (dst < hi)
        ds = dst[sel] - lo
        gs = gsrc[sel]
        vs = val[sel]
        blk = ds // P
        order = np.lexsort((gs, blk))
        ds, gs, vs, blk = ds[order], gs[order], vs[order], blk[order]
        bounds = np.searchsorted(blk, np.arange(n_blk + 1))
        perblk = []
        for b in range(n_blk):
            sl = slice(bounds[b], bounds[b + 1])
            gb, db, vb = gs[sl], ds[sl] - b * P, vs[sl]
            m = gb < SPLIT
            perblk.append((gb[m], db[m], vb[m],
                           gb[~m] - SPLIT, db[~m], vb[~m]))
        out.append(perblk)
    return out


def _pack(groups_rc, n_blk, R):
    nt = np.zeros((n_blk, R, 2), np.int64)
    for r in range(R):
        for c in range(N_CORES):
            for b in range(n_blk):
                g = groups_rc[r][c][b]
                nt[b, r, 0] = max(nt[b, r, 0], _ceil(len(g[0]), P))
                nt[b, r, 1] = max(nt[b, r, 1], _ceil(len(g[3]), P))
    tiles = []
    tmap = {}
    for b in range(n_blk):
        for r in range(R):
            tot = int(nt[b, r, 0] + nt[b, r, 1])
            k = 0
            for half in (0, 1):
                for j in range(int(nt[b, r, half])):
                    tmap[(b, r, half, j)] = len(tiles)
                    tiles.append((b, r, half, k == 0, k == tot - 1))
                    k += 1
    NT = len(tiles)
    src16 = np.zeros((N_CORES, NT, P), np.int16)
    dloc = np.full((N_CORES, NT, P), -1, np.int64)
    dval = np.zeros((N_CORES, NT, P), np.float32)
    for c in range(N_CORES):
        for b in range(n_blk):
            for r in range(R):
                g = groups_rc[r][c][b]
                for half in (0, 1):
                    sarr = g[0] if half == 0 else g[3]
                    darr = g[1] if half == 0 else g[4]
                    varr = g[2] if half == 0 else g[5]
                    for j in range(_ceil(len(sarr), P)):
                        t = tmap[(b, r, half, j)]
                        seg = slice(j * P, (j + 1) * P)
                        n = len(sarr[seg])
                        src16[c, t, :n] = sarr[seg]
                        dloc[c, t, :n] = darr[seg]
                        dval[c, t, :n] = varr[seg]
    return tiles, src16, dloc, dval


def _chunks_of_tiles(tiles):
    lo = [i for i, t in enumerate(tiles) if t[2] == 0]
    hi = [i for i, t in enumerate(tiles) if t[2] == 1]
    chunks = []
    for half, stream in ((0, lo), (1, hi)):
        for i in range(0, len(stream), CHUNK_TILES):
            chunks.append((half, stream[i:i + CHUNK_TILES]))
    chunks.sort(key=lambda ch: min(ch[1]))
    slot = {}
    for ci, (_, tl) in enumerate(chunks):
        for j, t in enumerate(tl):
            slot[t] = (ci, j)
    return chunks, slot


def _wrap_idx(src16, chunks):
    ncore = src16.shape[0]
    out = np.zeros((ncore, 128, max(1, len(chunks)) * (CHUNK // 16)), np.int16)
    for ci, (_, tl) in enumerate(chunks):
        flat = np.zeros((ncore, CHUNK), np.int16)
        for j, t in enumerate(tl):
            flat[:, j * P:(j + 1) * P] = src16[:, t, :]
        out[:, :16, ci * 64:(ci + 1) * 64] = flat.reshape(
            ncore, CHUNK // 16, 16).transpose(0, 2, 1)
    out[:, 16:, :] = np.tile(out[:, :16, :], (1, 7, 1))
    return out


def prepare(inputs, cfg):
    s = Struct()
    s.cfg = cfg
    N, R, NROW = cfg["N"], cfg["R"], cfg["NROW"]
    shard, rshard = N // N_CORES, NROW // N_CORES
    n_blk, n_rblk = _ceil(shard, P), _ceil(rshard, P)
    s.shard, s.rshard, s.n_blk, s.n_rblk = shard, rshard, n_blk, n_rblk
    assert AG_EDGES[-1] == shard and 4096 in AG_EDGES and 4096 * N_CORES == SPLIT

    es = np.asarray(inputs["edges_src"]).astype(np.int64)
    ed = np.asarray(inputs["edges_dst"]).astype(np.int64)

    g_main = []
    for r in range(R):
        deg_o = np.bincount(es[r], minlength=N).astype(np.float64)
        deg_i = np.bincount(ed[r], minlength=N).astype(np.float64)
        ns = np.where(deg_o > 0, deg_o, 1.0) ** -0.5
        nd = np.where(deg_i > 0, deg_i, 1.0) ** -0.5
        val = (ns[es[r]] * nd[ed[r]]).astype(np.float32)
        g_main.append(_bin_by_dst(_g_remap(es[r], shard), ed[r], val,
                                  shard, n_blk))
    s.tiles_e, src16_e, dloc_e, dval_e = _pack(g_main, n_blk, R)
    s.chunks_e, s.slot_e = _chunks_of_tiles(s.tiles_e)
    s.idx_e = _wrap_idx(src16_e, s.chunks_e)
    s.NT_e = len(s.tiles_e)
    s.n_ch_e = len(s.chunks_e)
    s.prefix_e = 0
    for half, _tl in s.chunks_e:
        if half != 0:
            break
        s.prefix_e += 1

    ridx = np.asarray(inputs["row_idx"]).astype(np.int64)
    rmask = np.asarray(inputs["row_mask"]).astype(bool)
    cnt = np.maximum(rmask.sum(1), 1).astype(np.float32)
    g_row = [[]]
    for c in range(N_CORES):
        lo = c * rshard
        rows = []
        for bb in range(n_rblk):
            i0 = lo + bb * P
            i1 = min(i0 + P, lo + rshard)
            ii, jj = np.nonzero(rmask[i0:i1])
            gs = _g_remap(ridx[i0:i1][ii, jj], shard)
            vv = (1.0 / cnt[i0 + ii]).astype(np.float32)
            order = np.argsort(gs, kind="stable")
            gs, ii2, vv = gs[order], ii[order], vv[order]
            m = gs < SPLIT
            rows.append((gs[m], ii2[m], vv[m],
                         gs[~m] - SPLIT, ii2[~m], vv[~m]))
        g_row[0].append(rows)
    s.tiles_r, src16_r, dloc_r, dval_r = _pack(g_row, n_rblk, 1)
    s.chunks_r, s.slot_r = _chunks_of_tiles(s.tiles_r)
    s.idx_r = _wrap_idx(src16_r, s.chunks_r)
    s.NT_r = len(s.tiles_r)
    s.n_ch_r = len(s.chunks_r)
    s.prefix_r = 0
    for half, _tl in s.chunks_r:
        if half != 0:
            break
        s.prefix_r += 1

    def build_S(NT, n_ch, slot, dloc, dval):
        out = []
        for c in range(N_CORES):
            Sh = np.zeros((P, n_ch * CHUNK), NPBF16)
            for ti in range(NT):
                ci, j = slot[ti]
                base = ci * CHUNK + j * P
                dl = dloc[c, ti]
                m = dl >= 0
                Sh[np.nonzero(m)[0], base + dl[m]] = \
                    dval[c, ti][m].astype(NPBF16)
            out.append(Sh)
        return out

    s.S_e = build_S(s.NT_e, s.n_ch_e, s.slot_e, dloc_e, dval_e)
    s.S_r = build_S(s.NT_r, s.n_ch_r, s.slot_r, dloc_r, dval_r)

    nf = np.asarray(inputs["node_feats"]).astype(np.float32)
    s.nfT_shards = []
    for c in range(N_CORES):
        m = np.zeros((n_blk * P, cfg["IN"]), np.float32)
        m[:shard] = nf[c * shard:(c + 1) * shard]
        s.nfT_shards.append(np.ascontiguousarray(m.T))
    return s


# ---------------------------------------------------------------------------
# device program
# ---------------------------------------------------------------------------
def build_program(s):
    cfg = s.cfg
    N, R = cfg["N"], cfg["R"]
    IN_D, HID, NCLS = cfg["IN"], cfg["HID"], cfg["NCLS"]
    n_blk, n_rblk, shard, rshard = s.n_blk, s.n_rblk, s.shard, s.rshard
    NQ = 4
    ag_start = AG_EDGES[:-1]
    ag_sizes = [AG_EDGES[i + 1] - AG_EDGES[i] for i in range(len(ag_start))]

    nc = bacc.Bacc("TRN2", target_bir_lowering=False, debug=False,
                   num_devices=N_CORES, num_swdge_queues=NQ)
    dp = nc.declare_dram_parameter
    t_nfT = dp("nfT", [IN_D, n_blk * P], F32, isOutput=False)
    t_Win = dp("W_in", [IN_D, HID], F32, isOutput=False)
    t_binrow = dp("binrow", [1, HID], F32, isOutput=False)
    t_W1 = dp("W1", [R * HID, HID], BF16, isOutput=False)
    t_W2 = dp("W2", [R * HID, HID], BF16, isOutput=False)
    t_bs1 = dp("bs1", [1, HID], BF16, isOutput=False)
    t_bs2 = dp("bs2", [1, HID], BF16, isOutput=False)
    t_Wm1 = dp("Wm1", [HID, HID], BF16, isOutput=False)
    t_Wm2 = dp("Wm2", [HID, HID], BF16, isOutput=False)
    t_Wm3 = dp("Wm3", [HID, NCLS], BF16, isOutput=False)
    t_bm1 = dp("bm1", [HID, 1], F32, isOutput=False)
    t_bm2 = dp("bm2", [HID, 1], F32, isOutput=False)
    t_bm3 = dp("bm3", [NCLS, 1], F32, isOutput=False)
    t_idx_e = dp("idx_e", list(s.idx_e.shape[1:]), I16, isOutput=False)
    t_idx_r = dp("idx_r", list(s.idx_r.shape[1:]), I16, isOutput=False)
    t_Se = dp("S_e", [128, s.n_ch_e * CHUNK], BF16, isOutput=False)
    t_Sr = dp("S_r", [128, s.n_ch_r * CHUNK], BF16, isOutput=False)
    t_out = dp("out", [NCLS, rshard], F32, isOutput=True)

    T = [nc.dram_tensor(f"T{i}", [N, HID], BF16, addr_space="Shared")
         for i in range(3)]
    Tsh = [nc.dram_tensor(f"T{i}sh", [shard, HID], BF16) for i in range(3)]

    with tile.TileContext(nc) as tc:
        import contextlib
        top = contextlib.ExitStack()
        kp = top.enter_context(tc.tile_pool(name="const", bufs=1))
        wp = top.enter_context(tc.tile_pool(name="weights", bufs=1))
        mp = top.enter_context(tc.tile_pool(name="meta", bufs=1))
        sb = top.enter_context(tc.tile_pool(name="sbwork", bufs=6))
        ttp = top.enter_context(tc.tile_pool(name="ttile", bufs=8))
        gp = top.enter_context(tc.tile_pool(name="gather", bufs=32))
        spool = top.enter_context(tc.tile_pool(name="spool", bufs=32))

        ones1 = kp.tile([1, 128], F32)
        nc.vector.memset(ones1[:], 1.0)
        ones1b = kp.tile([1, 128], BF16)
        nc.vector.memset(ones1b[:], 1.0)

        nfTsb = kp.tile([IN_D, n_blk * P], F32)
        nc.sync.dma_start(out=nfTsb[:], in_=t_nfT[:])

        Winsb = wp.tile([IN_D, HID], F32)
        nc.sync.dma_start(out=Winsb[:], in_=t_Win[:])
        binrow = wp.tile([1, HID], F32)
        nc.sync.dma_start(out=binrow[:], in_=t_binrow[:])
        W1sb = wp.tile([128, R * HID], BF16)
        W2sb = wp.tile([128, R * HID], BF16)
        for r in range(R):
            nc.sync.dma_start(out=W1sb[:HID, r * HID:(r + 1) * HID],
                              in_=t_W1[r * HID:(r + 1) * HID, :])
            nc.sync.dma_start(out=W2sb[:HID, r * HID:(r + 1) * HID],
                              in_=t_W2[r * HID:(r + 1) * HID, :])
        bs1row = wp.tile([1, HID], BF16)
        nc.sync.dma_start(out=bs1row[:], in_=t_bs1[:])
        bs2row = wp.tile([1, HID], BF16)
        nc.sync.dma_start(out=bs2row[:], in_=t_bs2[:])
        Wm1sb = wp.tile([HID, HID], BF16)
        nc.sync.dma_start(out=Wm1sb[:], in_=t_Wm1[:])
        Wm2sb = wp.tile([HID, HID], BF16)
        nc.sync.dma_start(out=Wm2sb[:], in_=t_Wm2[:])
        Wm3sb = wp.tile([HID, NCLS], BF16)
        nc.sync.dma_start(out=Wm3sb[:], in_=t_Wm3[:])
        bm1sb = wp.tile([HID, 1], F32)
        nc.sync.dma_start(out=bm1sb[:], in_=t_bm1[:])
        bm2sb = wp.tile([HID, 1], F32)
        nc.sync.dma_start(out=bm2sb[:], in_=t_bm2[:])
        bm3sb = wp.tile([NCLS, 1], F32)
        nc.sync.dma_start(out=bm3sb[:], in_=t_bm3[:])

        idxe_sb = mp.tile([128, s.idx_e.shape[2]], I16)
        nc.sync.dma_start(out=idxe_sb[:], in_=t_idx_e[:])
        idxr_sb = mp.tile([128, s.idx_r.shape[2]], I16)
        nc.sync.dma_start(out=idxr_sb[:], in_=t_idx_r[:])

        def write_table(l, b, src_ap, rows):
            tt = ttp.tile([128, HID], BF16, tag="tt")
            nc.scalar.activation(tt[:], src_ap,
                                 AF.Relu if l <= 1 else AF.Identity)
            nc.scalar.dma_start(out=Tsh[l][b * P:b * P + rows, :],
                                in_=tt[:rows, :])

        def ag_emit(l, k):
            nc.gpsimd.collective_compute(
                "AllGather", ALU.bypass,
                replica_groups=[list(range(N_CORES))],
                ins=[Tsh[l][ag_start[k]:ag_start[k] + ag_sizes[k], :]],
                outs=[T[l][N_CORES * ag_start[k]:
                           N_CORES * (ag_start[k] + ag_sizes[k]), :]])

        n_ag = len(ag_start)
        ag_blk = {_ceil(AG_EDGES[k + 1], P) - 1: k for k in range(n_ag)}


        # ---- lazy gather + S-load emitters (JIT, bounded lookahead) ------
        LOOKAHEAD = 7
        DEEP = 28

        def lazy_chunks(l, chunks, idx_sb, t_S):
            gtiles, stiles, state = {}, {}, {"next": 0}

            def ensure_to(tgt):
                tgt = min(tgt, len(chunks) - 1)
                while state["next"] <= tgt:
                    c2 = state["next"]
                    half, _tl = chunks[c2]
                    g = gp.tile([128, CHUNK_TILES, HID], BF16, tag="g")
                    srcT = T[l][0:SPLIT, :] if half == 0 else T[l][SPLIT:N, :]
                    nc.gpsimd.dma_gather(
                        out_ap=g[:], in_ap=srcT,
                        idxs_ap=idx_sb[:, c2 * 64:(c2 + 1) * 64],
                        num_idxs=CHUNK, num_idxs_reg=CHUNK, elem_size=HID,
                        queue_num=c2 % NQ)
                    gtiles[c2] = g
                    St = spool.tile([128, CHUNK], BF16, tag="S")
                    nc.sync.dma_start(out=St[:],
                                      in_=t_S[:, c2 * CHUNK:(c2 + 1) * CHUNK])
                    stiles[c2] = St
                    state["next"] += 1

            def ensure(ci):
                ensure_to(ci + LOOKAHEAD)
            return gtiles, stiles, ensure, ensure_to

        lzE0 = lazy_chunks(0, s.chunks_e, idxe_sb, t_Se)
        lzE1 = lazy_chunks(1, s.chunks_e, idxe_sb, t_Se)
        lzR = lazy_chunks(2, s.chunks_r, idxr_sb, t_Sr)

        # ---- phase 1: h0 = relu(x @ W_in + b_in) -------------------------
        with tc.tile_pool(name="ps_h0", bufs=4, space="PSUM") as pp:
            for b in range(n_blk):
                rows = min(P, shard - b * P)
                psh = pp.tile([128, HID], F32, tag="h0")
                nc.tensor.matmul(psh[:], lhsT=ones1[:], rhs=binrow[:],
                                 start=True, stop=False)
                nc.tensor.matmul(psh[:], lhsT=nfTsb[:, b * P:(b + 1) * P],
                                 rhs=Winsb[:], start=False, stop=True)
                write_table(0, b, psh[:], rows)
                k = ag_blk.get(b)
                if k is not None and k < n_ag - 1:
                    ag_emit(0, k)
        # half-0 gathers of layer 0 pre-issue before the last T0 chunk
        lzE0[3](s.prefix_e - 1)
        ag_emit(0, n_ag - 1)

        # ---- phases 2-3: RGCN layers ------------------------------------
        def run_layer(l, lz, lz_next, next_prefix):
            Wsb = W1sb if l == 0 else W2sb
            bsrow = bs1row if l == 0 else bs2row
            gtiles, stiles, ensure, ensure_to = lz
            with (
                tc.tile_pool(name=f"psx{l}", bufs=2, space="PSUM") as psxp,
                tc.tile_pool(name=f"ps2{l}", bufs=2, space="PSUM") as ps2p,
            ):
                def flush_block(b, psx, started, cur_ci):
                    rows = min(P, shard - b * P)
                    ps2 = ps2p.tile([128, 128], F32, tag="p2")
                    nc.tensor.matmul(ps2[:], lhsT=ones1b[:], rhs=bsrow[:],
                                     start=True, stop=False)
                    rlist = sorted(rr for (bb, rr) in started if bb == b)
                    for n_i, r in enumerate(rlist):
                        xs = sb.tile([128, 128], BF16, tag="xs")
                        nc.scalar.activation(xs[:], psx[:, r, :], AF.Copy)
                        nc.tensor.matmul(
                            ps2[:], lhsT=xs[:],
                            rhs=Wsb[:HID, r * HID:(r + 1) * HID],
                            start=False, stop=(n_i == len(rlist) - 1))
                    write_table(l + 1, b, ps2[:], rows)
                    k = ag_blk.get(b)
                    if k is not None and k < n_ag - 1:
                        # deep pre-issue so queued gathers drain during the
                        # collective (the CC blocks the gpsimd issue stream)
                        ensure_to(cur_ci + DEEP)
                        ag_emit(l + 1, k)

                cur_blk, psx, started, cur_ci = -1, None, set(), 0
                for ti, (b, r, half, first, last) in enumerate(s.tiles_e):
                    ci, j = s.slot_e[ti]
                    if b != cur_blk:
                        if cur_blk >= 0:
                            flush_block(cur_blk, psx, started, cur_ci)
                        cur_blk = b
                        psx = psxp.tile([128, R, 128], F32, tag="psx")
                        started = set()
                    ensure(ci)
                    cur_ci = max(cur_ci, ci)
                    nc.tensor.matmul(
                        psx[:, r, :], lhsT=gtiles[ci][:, j, :],
                        rhs=stiles[ci][:, j * P:(j + 1) * P],
                        start=(b, r) not in started, stop=last)
                    started.add((b, r))
                if cur_blk >= 0:
                    flush_block(cur_blk, psx, started, cur_ci)
                # pre-issue next phase's half-0 gathers, then last AG chunk
                lz_next[3](next_prefix - 1)
                ag_emit(l + 1, n_ag - 1)

        run_layer(0, lzE0, lzE1, s.prefix_e)
        run_layer(1, lzE1, lzR, s.prefix_r)

        # ---- phase 4: rows + MLP ----------------------------------------
        with (
            tc.tile_pool(name="psr", bufs=2, space="PSUM") as psrp,
            tc.tile_pool(name="psm", bufs=2, space="PSUM") as psmp,
        ):
            gtiles, stiles, ensure, ensure_to = lzR

            def flush_rblock(bb, psr):
                xr = sb.tile([128, 128], BF16, tag="xr")
                nc.scalar.activation(xr[:], psr[:], AF.Copy)
                pm = psmp.tile([128, 128], F32, tag="pm")
                nc.tensor.matmul(pm[:], lhsT=Wm1sb[:], rhs=xr[:], start=True,
                                 stop=True)
                a1 = sb.tile([128, 128], BF16, tag="a1")
                nc.scalar.activation(a1[:], pm[:], AF.Relu, bias=bm1sb[:])
                pm2 = psmp.tile([128, 128], F32, tag="pm")
                nc.tensor.matmul(pm2[:], lhsT=Wm2sb[:], rhs=a1[:], start=True,
                                 stop=True)
                a2 = sb.tile([128, 128], BF16, tag="a2")
                nc.scalar.activation(a2[:], pm2[:], AF.Relu, bias=bm2sb[:])
                pm3 = psmp.tile([NCLS, 128], F32, tag="pm3")
                nc.tensor.matmul(pm3[:], lhsT=Wm3sb[:], rhs=a2[:], start=True,
                                 stop=True)
                ot = sb.tile([NCLS, 128], F32, tag="ot")
                nc.scalar.activation(ot[:], pm3[:], AF.Identity, bias=bm3sb[:])
                cols = min(P, rshard - bb * P)
                nc.scalar.dma_start(out=t_out[:, bb * P:bb * P + cols],
                                    in_=ot[:, :cols])

            r_byblk = {}
            for ti, (bb, r0, half, first, last) in enumerate(s.tiles_r):
                r_byblk.setdefault(bb, []).append((ti, last))
            for bb in range(n_rblk):
                psr = psrp.tile([128, 128], F32, tag="psrT")
                tl = r_byblk.get(bb, [])
                if not tl:
                    nc.vector.memset(psr[:], 0.0)
                rstarted = False
                for (ti, last) in tl:
                    ci, j = s.slot_r[ti]
                    ensure(ci)
                    nc.tensor.matmul(psr[:], lhsT=gtiles[ci][:, j, :],
                                     rhs=stiles[ci][:, j * P:(j + 1) * P],
                                     start=not rstarted, stop=last)
                    rstarted = True
                flush_rblock(bb, psr)

        top.close()

    nc.compile()
    return nc


# ---------------------------------------------------------------------------
# entry point
# ---------------------------------------------------------------------------
def make_in_maps(inputs, s):
    cfg = s.cfg
    HID = cfg["HID"]
    W1 = np.asarray(inputs["W1"], np.float32).reshape(-1, HID).astype(NPBF16)
    W2 = np.asarray(inputs["W2"], np.float32).reshape(-1, HID).astype(NPBF16)
    bs1 = np.asarray(inputs["b1"], np.float32).sum(0).reshape(1, -1).astype(NPBF16)
    bs2 = np.asarray(inputs["b2"], np.float32).sum(0).reshape(1, -1).astype(NPBF16)
    in_maps = []
    for c in range(N_CORES):
        m = {
            "nfT": s.nfT_shards[c],
            "W_in": np.asarray(inputs["W_in"], np.float32),
            "binrow": np.asarray(inputs["b_in"], np.float32).reshape(1, -1),
            "W1": W1, "W2": W2, "bs1": bs1, "bs2": bs2,
            "Wm1": np.asarray(inputs["Wm1"], np.float32).astype(NPBF16),
            "Wm2": np.asarray(inputs["Wm2"], np.float32).astype(NPBF16),
            "Wm3": np.asarray(inputs["Wm3"], np.float32).astype(NPBF16),
            "bm1": np.asarray(inputs["bm1"], np.float32).reshape(-1, 1),
            "bm2": np.asarray(inputs["bm2"], np.float32).reshape(-1, 1),
            "bm3": np.asarray(inputs["bm3"], np.float32).reshape(-1, 1),
            "idx_e": np.ascontiguousarray(s.idx_e[c]),
            "idx_r": np.ascontiguousarray(s.idx_r[c]),
            "S_e": s.S_e[c],
            "S_r": s.S_r[c],
        }
        in_maps.append(m)
    return in_maps


def run(inputs, cfg, runner=None):
    s = prepare(inputs, cfg)
    nc = build_program(s)
    in_maps = make_in_maps(inputs, s)
    if runner is None:
        res = bass_utils.run_bass_kernel_spmd(nc, in_maps,
                                              core_ids=list(range(N_CORES)))
    else:
        res = runner(nc, in_maps)
    out = np.concatenate(
        [res.results[c]["out"][:, :s.rshard].T for c in range(N_CORES)],
        axis=0)
    return out.astype(np.float32), s, nc, res


def kernel(node_feats, edges_src, edges_dst, row_idx, row_mask,
           W_in, b_in, W1, b1, W2, b2, Wm1, bm1, Wm2, bm2, Wm3, bm3):
    cfg = dict(N=38000, R=8, NROW=60000, F=19, IN=64, HID=128, NCLS=10)
    inputs = dict(node_feats=node_feats, edges_src=edges_src,
                  edges_dst=edges_dst, row_idx=row_idx, row_mask=row_mask,
                  W_in=W_in, b_in=b_in, W1=W1, b1=b1, W2=W2, b2=b2,
                  Wm1=Wm1, bm1=bm1, Wm2=Wm2, bm2=bm2, Wm3=Wm3, bm3=bm3)
    out, _, _, _ = run(inputs, cfg)
    return out

